# revision 21
# baseline (speedup 1.0000x reference)
"""Trainium2 Bass kernel for a binarized VGG-style CNN (BNN).

Data-parallel over 8 NeuronCores: batch 256 -> 32 per core; weights
replicated.  Host prep binarizes weights/activations and folds batch-
norms so every matmul is exact (products of +-1 / {0,2} are exact in
fp8/bf16/fp16, accumulation is fp32 in PSUM):

  conv1            : host im2col of sign(x), K=27, fp8, exact
  conv2/conv3      : fp8 {0,2} x fp8 +-1, 9-tap shifted GEMMs; vertical
                     tap pairs run in DoubleRow fp8 mode (row stride
                     padded to 16B so the j-step is legal); conv3 packs
                     3 batches per chunk by "rolling" rows across the
                     batch dim (seam rows computed then discarded)
  conv5            : im2col (X5col) fp8 DoubleRow over its 2 k-tiles
  activations      : {0,2} = sign+1 via one DVE is_ge per evict with
                     host-folded per-channel thresholds (conv3/conv5
                     thresholds are exact integers since actq precedes
                     the bn there)
  conv4            : input is bn3(sign); folded into weights w4*inv3,
                     split hi/lo fp16 (2 exact passes ~ 22 bits)
  conv6            : input bn5(sign) built on-chip in fp32, split in 2
                     fp16 parts, M-stacked (64 rows) flipped GEMM,
                     partials summed on DVE, PE-transposed back
  final evicts     : ScalarE Sign(x*scale + bias) (exact fp32 affine)

The PE is pre-warmed with dummy matmuls during the input DMA wait so
the HAM clock gate opens before conv1; bulky weight DMAs ride the
gpsimd SWDGE queues so the input im2col owns the HWDGE queue.
"""

import numpy as np
import ml_dtypes

BF = ml_dtypes.bfloat16
F16 = np.float16
F8 = ml_dtypes.float8_e4m3
NCORES = 8
B = 32
TAPS = [(kh, kw) for kh in range(3) for kw in range(3)]

# ----------------------------------------------------------------------------
# host-side prep
# ----------------------------------------------------------------------------

def _sign(a):
    return np.where(a >= 0, np.float32(1.0), np.float32(-1.0)).astype(np.float32)


def _inv_shift(bn):
    g, b, m, v = (bn[i].astype(np.float32) for i in range(4))
    inv = (g / np.sqrt(v + np.float32(1e-5))).astype(np.float32)
    shift = (b - m * inv).astype(np.float32)
    return inv, shift


def _split2_f16(w64):
    hi = w64.astype(F16)
    r = w64 - hi.astype(np.float64)
    lo = r.astype(F16)
    return hi, lo


def _prep_shared(i):
    d = {}
    inv1, shift1 = _inv_shift(i['bn1'])
    inv2, shift2 = _inv_shift(i['bn2'])
    inv3, shift3 = _inv_shift(i['bn3'])
    inv4, shift4 = _inv_shift(i['bn4'])
    inv5, shift5 = _inv_shift(i['bn5'])
    inv6, shift6 = _inv_shift(i['bn6'])
    inv7, shift7 = _inv_shift(i['bn7'])
    inv8, shift8 = _inv_shift(i['bn8'])
    inv9, shift9 = _inv_shift(i['bn9'])
    f64 = np.float64

    w1q = _sign(i['w1'])
    w1l = np.empty((27, 128), np.float32)
    for t, (kh, kw) in enumerate(TAPS):
        w1l[t * 3:(t + 1) * 3, :] = w1q[:, :, kh, kw].T
    d['w1d'] = w1l.astype(F8)

    # conv2 weights: per kw, DR pair (kh=0,1) interleaved + single kh=2
    #   col = kw*384 + j*128 + oc   (j = kh for the pair)
    #   col = kw*384 + 256 + oc     (kh = 2)
    w2q = _sign(i['w2'])
    w2l = np.empty((128, 3 * 384), np.float32)
    for kw in range(3):
        # DoubleRowSwInterleave storage: [A127,B127,A126,B126,...,A0,B0]
        # where A = kh0 weights, B = kh1 weights, column-reversed.
        for m in range(128):
            w2l[:, kw * 384 + 2 * (127 - m)] = w2q[m, :, 0, kw]
            w2l[:, kw * 384 + 2 * (127 - m) + 1] = w2q[m, :, 1, kw]
        w2l[:, kw * 384 + 256: kw * 384 + 384] = w2q[:, :, 2, kw].T
    d['w2d'] = w2l.astype(F8)

    # conv3 weights: plain lhsT per (tap, oct): col = (t*2+o)*128 + oc
    w3q = _sign(i['w3'])
    w3l = np.empty((128, 9 * 256), np.float32)
    for t, (kh, kw) in enumerate(TAPS):
        for o in range(2):
            w3l[:, (t * 2 + o) * 128:(t * 2 + o + 1) * 128] = \
                w3q[o * 128:(o + 1) * 128, :, kh, kw].T
    d['w3d'] = w3l.astype(F8)

    # conv4: folded weights w4q*inv3, 2-way fp16 split
    w4q = _sign(i['w4'])
    wt4 = w4q.astype(f64) * inv3.astype(f64)[None, :, None, None]
    hi, lo = _split2_f16(wt4)
    w4l = np.empty((256, 2 * 9 * 256), F16)
    for p, part in enumerate((hi, lo)):
        for t, (kh, kw) in enumerate(TAPS):
            for o in range(2):
                col = ((p * 9 + t) * 2 + o) * 128
                w4l[:, col:col + 128] = part[o * 128:(o + 1) * 128, :, kh, kw].T
    d['w4d'] = w4l

    # conv5 weights: DR over the 2 k-tiles: [c, j(kt), m] per (tap, oct)
    #   col = (t*4 + o)*256 + j*128 + m
    w5q = _sign(i['w5'])
    w5l = np.empty((128, 9 * 4 * 256), np.float32)
    for t, (kh, kw) in enumerate(TAPS):
        for o in range(4):
            base = (t * 4 + o) * 256
            for m in range(128):
                w5l[:, base + 2 * (127 - m)] = w5q[o * 128 + m, 0:128, kh, kw]
                w5l[:, base + 2 * (127 - m) + 1] = w5q[o * 128 + m, 128:256, kh, kw]
    d['w5d'] = w5l.astype(F8)

    w6q = _sign(i['w6'])
    w6r = np.empty((512, 9 * 512), np.float32)
    for t, (kh, kw) in enumerate(TAPS):
        w6r[:, t * 512:(t + 1) * 512] = w6q[:, :, kh, kw].T
    d['w6d'] = w6r.astype(BF)

    f1q = _sign(i['fc1'])
    f2q = _sign(i['fc2'])
    f3q = _sign(i['fc3'])
    d['f1d'] = f1q.T.copy().astype(BF)
    d['f2d'] = f2q.T.copy().astype(BF)
    d['f3d'] = f3q.T.copy().astype(BF)
    t7 = (-shift7.astype(f64) / inv7.astype(f64)).astype(np.float32)
    t8 = (f2q.sum(axis=1).astype(f64) - shift8.astype(f64) / inv8.astype(f64)
          ).astype(np.float32)
    t9 = (f3q.sum(axis=1).astype(f64) - shift9.astype(f64) / inv9.astype(f64)
          ).astype(np.float32)

    # thresholds ({0,2} propagation: psum = true_conv + rowsum(wq))
    rs2 = w2q.sum(axis=(1, 2, 3)).astype(f64)
    rs3 = w3q.sum(axis=(1, 2, 3)).astype(f64)
    rs5 = w5q.sum(axis=(1, 2, 3)).astype(f64)
    t1 = inv1  # ACT Sign scale
    t1b = shift1  # ACT Sign bias
    t2 = (-shift2.astype(f64) / inv2.astype(f64)).astype(np.float32)
    t3 = rs3.astype(np.float32)
    split_sum = hi.astype(f64) + lo.astype(f64)
    rowsum_split = split_sum.sum(axis=(1, 2, 3))
    shift3_term = w4q.sum(axis=(2, 3)).astype(f64) @ shift3.astype(f64)
    t4 = (rowsum_split - shift3_term - shift4.astype(f64) / inv4.astype(f64)
          ).astype(np.float32)
    t5 = rs5.astype(np.float32)

    bnv = np.zeros((128, 72), np.float32)
    bnv[:, 0] = t1
    bnv[:, 68] = t1b
    bnv[:, 1] = t2
    for o in range(2):
        bnv[:, 2 + o] = t3[o * 128:(o + 1) * 128]
        bnv[:, 4 + o] = t4[o * 128:(o + 1) * 128]
    # conv6 input X6 takes two values per channel: a = fp32(inv5+shift5)
    # (s5=+1) and b = fp32(-inv5+shift5).  Split each into fp16 hi+lo on
    # the host; on-chip the evict selects via (psum>=t5)*(hi_a-hi_b)+hi_b
    # (all arithmetic exact: fp16 diffs are exact in fp32, and the final
    # add reconstructs the fp16 value exactly).
    a = (inv5 + shift5).astype(np.float32)
    bq = (-inv5 + shift5).astype(np.float32)
    fa = a.astype(F16); fb = bq.astype(F16)
    la = (a.astype(f64) - fa.astype(f64)).astype(F16)
    lb = (bq.astype(f64) - fb.astype(f64)).astype(F16)
    dhi = (fa.astype(np.float32) - fb.astype(np.float32))
    dlo = (la.astype(np.float32) - lb.astype(np.float32))
    for k in range(4):
        bnv[:, 6 + k] = t5[k * 128:(k + 1) * 128]
        bnv[:, 10 + k] = dhi[k * 128:(k + 1) * 128]
        bnv[:, 14 + k] = fb.astype(np.float32)[k * 128:(k + 1) * 128]
        bnv[:, 60 + k] = dlo[k * 128:(k + 1) * 128]
        bnv[:, 64 + k] = lb.astype(np.float32)[k * 128:(k + 1) * 128]
        bnv[:, 18 + k] = inv6[k * 128:(k + 1) * 128]
        bnv[:, 22 + k] = shift6[k * 128:(k + 1) * 128]
    for o in range(8):
        bnv[:, 26 + o] = t7[o * 128:(o + 1) * 128]
        bnv[:, 42 + o] = t8[o * 128:(o + 1) * 128]
    bnv[:10, 58] = t9
    d['bnd'] = bnv
    return d


def _prep_xim(x_shard):
    """sign(x) im2col: [32,3,32,32] -> [27, 32*900] fp8."""
    xs = _sign(x_shard)
    arr = np.stack([xs[:, :, kh:kh + 30, kw:kw + 30] for (kh, kw) in TAPS], 0)
    return arr.transpose(0, 2, 1, 3, 4).reshape(27, 32 * 900).astype(F8)


# ----------------------------------------------------------------------------
# bass kernel build
# ----------------------------------------------------------------------------

def _build_nc():
    import concourse.bass as bass
    import concourse.bacc as bacc
    import concourse.tile as tile
    from concourse import mybir
    from concourse.masks import make_identity
    from contextlib import ExitStack

    bf16 = mybir.dt.bfloat16
    fp16 = mybir.dt.float16
    fp8 = mybir.dt.float8e4
    f32 = mybir.dt.float32
    A = mybir.AluOpType
    SIGN = mybir.ActivationFunctionType.Sign
    COPY = mybir.ActivationFunctionType.Identity
    DR = mybir.MatmulPerfMode.DoubleRow
    DRSW = mybir.MatmulPerfMode.DoubleRowSwInterleave

    nc = bacc.Bacc("TRN2", target_bir_lowering=False, debug=False)

    xim = nc.dram_tensor("xim", [27, B * 900], fp8, kind="ExternalInput").ap()
    w1d = nc.dram_tensor("w1d", [27, 128], fp8, kind="ExternalInput").ap()
    w2d = nc.dram_tensor("w2d", [128, 3 * 384], fp8, kind="ExternalInput").ap()
    w3d = nc.dram_tensor("w3d", [128, 9 * 256], fp8, kind="ExternalInput").ap()
    w4d = nc.dram_tensor("w4d", [256, 18 * 256], fp16, kind="ExternalInput").ap()
    w5d = nc.dram_tensor("w5d", [128, 9 * 4 * 256], fp8, kind="ExternalInput").ap()
    w6d = nc.dram_tensor("w6d", [512, 9 * 512], bf16, kind="ExternalInput").ap()
    f1d = nc.dram_tensor("f1d", [512, 1024], bf16, kind="ExternalInput").ap()
    f2d = nc.dram_tensor("f2d", [1024, 1024], bf16, kind="ExternalInput").ap()
    f3d = nc.dram_tensor("f3d", [1024, 10], bf16, kind="ExternalInput").ap()
    bnd = nc.dram_tensor("bnd", [128, 72], f32, kind="ExternalInput").ap()
    od = nc.dram_tensor("od", [10, B], f32, kind="ExternalOutput").ap()

    def sub_ap(base, extra_off, pairs):
        return bass.AP(base.tensor, base.offset + extra_off,
                       [list(base.ap[0])] + [list(p) for p in pairs])

    with tile.TileContext(nc) as tc, ExitStack() as top:
        const = top.enter_context(tc.tile_pool(name="const", bufs=1))
        bnv = const.tile([128, 72], f32, tag="bnv")
        nc.sync.dma_start(bnv[:], bnd)
        ident = const.tile([32, 32], f32, tag="ident")
        make_identity(nc, ident)

        def sc(c):
            return bnv[:, c:c + 1]

        acts = top.enter_context(tc.tile_pool(name="acts", bufs=1))
        h1 = acts.tile([128, B * 30 * 32], fp8, tag="h1")       # rows padded 30->32
        h2 = acts.tile([128, B * 14 * 16], fp8, tag="h2")       # rows padded 14->16
        s3 = [acts.tile([128, B * 144], fp8, name=f"s3_{k}", tag=f"s3_{k}")
              for k in range(2)]
        h4 = [acts.tile([128, B * 25], fp8, name=f"h4_{k}", tag=f"h4_{k}")
              for k in range(2)]
        x5c = acts.tile([128, 2, 9, B * 9], fp8, tag="x5c")
        x6s = [acts.tile([128, 9, 2, B], fp16, name=f"x6s_{k}", tag=f"x6s_{k}")
               for k in range(4)]
        h6 = [acts.tile([128, B], bf16, name=f"h6_{k}", tag=f"h6_{k}")
              for k in range(4)]
        h7 = [acts.tile([128, B], bf16, name=f"h7_{k}", tag=f"h7_{k}")
              for k in range(8)]
        h8 = [acts.tile([128, B], bf16, name=f"h8_{k}", tag=f"h8_{k}")
              for k in range(8)]

        # ---------------------------------------- PE pre-warm during DMA wait
        warm = top.enter_context(tc.tile_pool(name="warm", bufs=1))
        wz = warm.tile([128, 640], bf16, tag="wz")
        nc.gpsimd.memset(wz[:], 0.0)
        pwp_ctx = ExitStack()
        pwp = pwp_ctx.enter_context(tc.tile_pool(name="pswarm", bufs=1, space="PSUM"))
        pw = pwp.tile([128, 512], f32, tag="pw")
        _warm_n = [0]

        def dummy_mm(n=1, N=128):
            for _ in range(n):
                nc.tensor.matmul(pw[:, :N], wz[:, 0:128], wz[:, 128:128 + N],
                                 start=(_warm_n[0] == 0), stop=False,
                                 skip_group_check=True)
                _warm_n[0] += 1

        dummy_mm(26, N=512)
        pwp_ctx.close()

        # ---------------------------------------- weight DMAs
        # input path (sync / HWDGE queue): bnv, xim chunks, w1..w3
        # bulk weights (gpsimd / SWDGE queues): w4..w6, fc
        w2pool = top.enter_context(tc.tile_pool(name="w2pool", bufs=1))
        w3pool = top.enter_context(tc.tile_pool(name="w3pool", bufs=1))

        # -------------------------------- conv1 + conv2 (interleaved emission)
        # conv1 is K=27: on its own the PE activity monitor sees it as
        # near-idle and throttles the clock.  Emitting conv1(b) interleaved
        # with conv2(b-1) keeps dense K=128 DR work in the PE stream, so
        # the whole phase runs at full clock.
        w4pool = top.enter_context(tc.tile_pool(name="w4pool", bufs=1))
        w4 = [w4pool.tile([128, 18 * 256], fp16, name=f"w4_{k}", tag=f"w4_{k}")
              for k in range(2)]
        _w4dmas = [nc.gpsimd.dma_start(w4[k][:], w4d[k * 128:(k + 1) * 128, :])
                   for k in range(2)]
        with ExitStack() as ctx:
            p01 = ctx.enter_context(tc.tile_pool(name="p01", bufs=1))
            ximt = p01.tile([27, B * 900], fp8, tag="ximt")
            xdma = None
            for q in range(8):
                s = q * 4 * 900
                e = (q + 1) * 4 * 900
                eng = nc.sync if q % 2 == 0 else nc.scalar
                xdma = eng.dma_start(ximt[:, s:e], xim[:, s:e])
            w1 = p01.tile([27, 128], fp8, tag="w1")
            nc.sync.dma_start(w1[:], w1d)
            w2 = w2pool.tile([128, 3 * 384], fp8, tag="w2")
            nc.sync.dma_start(w2[:], w2d)
            w3 = w3pool.tile([128, 9 * 256], fp8, tag="w3")
            nc.sync.dma_start(w3[:], w3d)
            from concourse.tile_rust import add_dep_helper
            for _wd in _w4dmas:
                add_dep_helper(_wd.ins, xdma.ins, sync=True,
                               reason="bulk weight DMA waits for input im2col")
            pp1 = ctx.enter_context(tc.tile_pool(name="ps1", bufs=2, space="PSUM"))
            sp = ctx.enter_context(tc.tile_pool(name="sc2", bufs=4))
            pp2 = ctx.enter_context(tc.tile_pool(name="ps2", bufs=6, space="PSUM"))
            xr = ximt[:].rearrange("p (b h w) -> p b h w", b=B, h=30, w=30)
            h1v = h1[:].rearrange("p (b h w) -> p b h w", b=B, h=30, w=32)
            nc.gpsimd.memset(h1v[:, :, :, 30:32], 0.0)
            h2v = h2[:].rearrange("p (b h w) -> p b h w", b=B, h=14, w=16)
            NR = 13 * 32 + 28

            def conv1_emit(b):
                for rh in range(2):
                    ps = pp1.tile([128, 450], f32, tag="ps1t")
                    nc.tensor.matmul(ps[:], w1[:], xr[:, b, rh * 15:rh * 15 + 15, :],
                                     start=True, stop=True)
                    psv = ps[:].rearrange("p (r w) -> p r w", r=15, w=30)
                    nc.scalar.activation(
                        h1v[:, b, rh * 15:(rh + 1) * 15, 0:30],
                        psv, SIGN, bias=sc(68), scale=sc(0))

            def conv2_emit(bpair):
                chunks = [(b, rh) for b in bpair for rh in range(2)]
                pss = [pp2.tile([128, NR], f32, name=f"ps_{b}_{rh}", tag="ps")
                       for (b, rh) in chunks]
                for kw in range(3):
                    for ci, (b, rh) in enumerate(chunks):
                        rhs = sub_ap(h1[:], b * 960 + rh * 14 * 32 + kw,
                                     [[32, 2], [1, NR]])
                        nc.tensor.matmul(
                            pss[ci][:], w2[:, kw * 384:kw * 384 + 256],
                            rhs, start=(kw == 0), stop=False, perf_mode=DRSW)
                    for ci, (b, rh) in enumerate(chunks):
                        rhs2 = sub_ap(h1[:], b * 960 + (rh * 14 + 2) * 32 + kw,
                                      [[1, NR]])
                        nc.tensor.matmul(
                            pss[ci][:], w2[:, kw * 384 + 256:kw * 384 + 384],
                            rhs2, start=False, stop=(kw == 2))
                for ci, (b, rh) in enumerate(chunks):
                    ps = pss[ci]
                    tmp = sp.tile([128, 392], bf16, tag="tmp")
                    psv = sub_ap(ps[:], 0, [[32, 14], [1, 28]])
                    tmpv = tmp[:].rearrange("p (r w) -> p r w", r=14, w=28)
                    nc.vector.tensor_scalar(tmpv, psv, sc(1), 2.0, A.is_ge, A.mult)
                    tr = tmp[:].rearrange("p (r rr w ww) -> p r rr w ww",
                                          r=7, rr=2, w=14, ww=2)
                    m1 = sp.tile([128, 98], bf16, tag="m1")
                    m1r = m1[:].rearrange("p (r w) -> p r w", r=7, w=14)
                    m2 = sp.tile([128, 98], bf16, tag="m2")
                    m2r = m2[:].rearrange("p (r w) -> p r w", r=7, w=14)
                    nc.vector.tensor_tensor(m1r, tr[:, :, 0, :, 0], tr[:, :, 0, :, 1], A.max)
                    nc.vector.tensor_tensor(m2r, tr[:, :, 1, :, 0], tr[:, :, 1, :, 1], A.max)
                    nc.vector.tensor_tensor(
                        h2v[:, b, rh * 7:(rh + 1) * 7, 0:14], m1r, m2r, A.max)

            for bp in range(B // 2):
                conv1_emit(2 * bp)
                conv1_emit(2 * bp + 1)
                if bp >= 1:
                    conv2_emit((2 * bp - 2, 2 * bp - 1))
            conv2_emit((B - 2, B - 1))

        # ------------------------------------------------ conv3 -> s3 {0,2}
        bgrp3 = [(b0, min(b0 + 3, B)) for b0 in range(0, B, 3)]
        with ExitStack() as ctx:
            w5pool = top.enter_context(tc.tile_pool(name="w5pool", bufs=1))
            w5 = w5pool.tile([128, 9 * 4 * 256], fp8, tag="w5")
            nc.gpsimd.dma_start(w5[:], w5d)
            pp = ctx.enter_context(tc.tile_pool(name="ps3", bufs=6, space="PSUM"))
            h2v3 = h2[:].rearrange("p (b h w) -> p b h w", b=B, h=14, w=16)
            for o in range(2):
                for (b0, b1) in bgrp3:
                    nb = b1 - b0
                    ps = pp.tile([128, 3 * 144], f32, tag="ps")
                    for t, (kh, kw) in enumerate(TAPS):
                        nc.tensor.matmul(
                            ps[:, :nb * 144],
                            w3[:, (t * 2 + o) * 128:(t * 2 + o + 1) * 128],
                            h2v3[:, b0:b1, kh:kh + 12, kw:kw + 12],
                            start=(t == 0), stop=(t == 8))
                    nc.vector.tensor_scalar(
                        s3[o][:, b0 * 144:b1 * 144], ps[:, :nb * 144],
                        sc(2 + o), 2.0, A.is_ge, A.mult)

        # ------------------------------------------------ conv4 (2-pass fp16)
        bgrp4 = [(b0, min(b0 + 5, B)) for b0 in range(0, B, 5)]
        with ExitStack() as ctx:
            w6pool = top.enter_context(tc.tile_pool(name="w6pool", bufs=1))
            w6 = [w6pool.tile([128, 9 * 512], bf16, name=f"w6_{k}", tag=f"w6_{k}")
                  for k in range(4)]
            for k in range(4):
                nc.gpsimd.dma_start(w6[k][:], w6d[k * 128:(k + 1) * 128, :])
            fcpool = top.enter_context(tc.tile_pool(name="fcpool", bufs=1))
            f1 = [fcpool.tile([128, 1024], bf16, name=f"f1_{k}", tag=f"f1_{k}")
                  for k in range(4)]
            f2 = [fcpool.tile([128, 1024], bf16, name=f"f2_{k}", tag=f"f2_{k}")
                  for k in range(8)]
            f3 = [fcpool.tile([128, 10], bf16, name=f"f3_{k}", tag=f"f3_{k}")
                  for k in range(8)]
            for k in range(4):
                nc.gpsimd.dma_start(f1[k][:], f1d[k * 128:(k + 1) * 128, :])
            for k in range(8):
                nc.gpsimd.dma_start(f2[k][:], f2d[k * 128:(k + 1) * 128, :])
            for k in range(8):
                nc.gpsimd.dma_start(f3[k][:], f3d[k * 128:(k + 1) * 128, :])
            sp = ctx.enter_context(tc.tile_pool(name="sc4", bufs=4))
            pp = ctx.enter_context(tc.tile_pool(name="ps4", bufs=6, space="PSUM"))
            s3v = [s3[k][:].rearrange("p (b h w) -> p b h w", b=B, h=12, w=12)
                   for k in range(2)]
            h4v = [h4[k][:].rearrange("p (b h w) -> p b h w", b=B, h=5, w=5)
                   for k in range(2)]
            for o in range(2):
                for (b0, b1) in bgrp4:
                    nb = b1 - b0
                    n = nb * 100
                    ps = pp.tile([128, 500], f32, tag="ps")
                    first = True
                    for p in range(2):
                        for t, (kh, kw) in enumerate(TAPS):
                            for k in range(2):
                                col = ((p * 9 + t) * 2 + o) * 128
                                nc.tensor.matmul(
                                    ps[:, :n], w4[k][:, col:col + 128],
                                    s3v[k][:, b0:b1, kh:kh + 10, kw:kw + 10],
                                    start=first, stop=(p == 1 and t == 8 and k == 1))
                                first = False
                    tmp = sp.tile([128, 500], bf16, tag="tmp")
                    nc.vector.tensor_scalar(tmp[:, :n], ps[:, :n],
                                            sc(4 + o), 2.0, A.is_ge, A.mult)
                    tr = tmp[:, :n].rearrange("p (b r rr w ww) -> p b r rr w ww",
                                              b=nb, r=5, rr=2, w=5, ww=2)
                    m1 = sp.tile([128, 125], bf16, tag="m1")
                    m1r = m1[:, :nb * 25].rearrange("p (b r w) -> p b r w", b=nb, r=5, w=5)
                    m2 = sp.tile([128, 125], bf16, tag="m2")
                    m2r = m2[:, :nb * 25].rearrange("p (b r w) -> p b r w", b=nb, r=5, w=5)
                    nc.vector.tensor_tensor(m1r, tr[:, :, :, 0, :, 0], tr[:, :, :, 0, :, 1], A.max)
                    nc.vector.tensor_tensor(m2r, tr[:, :, :, 1, :, 0], tr[:, :, :, 1, :, 1], A.max)
                    nc.vector.tensor_tensor(h4v[o][:, b0:b1, :, :], m1r, m2r, A.max)

        # ------------------------------------------------ conv5 (X5col fp8 DR)
        with ExitStack() as ctx:
            sp = ctx.enter_context(tc.tile_pool(name="sc5", bufs=2))
            pp = ctx.enter_context(tc.tile_pool(name="ps5", bufs=4, space="PSUM"))
            h4r = [h4[k][:].rearrange("p (b h w) -> p b h w", b=B, h=5, w=5)
                   for k in range(2)]
            for j in range(2):
                for t, (kh, kw) in enumerate(TAPS):
                    ov = x5c[:, j, t, :].rearrange("p (b h w) -> p b h w",
                                                   b=B, h=3, w=3)
                    nc.vector.tensor_copy(ov, h4r[j][:, :, kh:kh + 3, kw:kw + 3])
            pp6 = ctx.enter_context(tc.tile_pool(name="ps6", bufs=1, space="PSUM"))
            pt = ctx.enter_context(tc.tile_pool(name="ps6t", bufs=2, space="PSUM"))
            ps6 = pp6.tile([64, 512], f32, tag="ps6")

            def conv5_oct(o):
                ps = pp.tile([128, B * 9], f32, tag="ps")
                for t in range(9):
                    nc.tensor.matmul(
                        ps[:], w5[:, (t * 4 + o) * 256:(t * 4 + o + 1) * 256],
                        x5c[:, :, t, :], start=(t == 0), stop=(t == 8),
                        perf_mode=DRSW)
                for p, (dc, bc) in enumerate(((10, 14), (60, 64))):
                    sel = sp.tile([128, B * 9], f32, tag="sel")
                    nc.vector.tensor_scalar(sel[:], ps[:], sc(6 + o), sc(dc + o),
                                            A.is_ge, A.mult)
                    sv = sel[:].rearrange("p (b t) -> p b t", b=B, t=9)
                    ov = x6s[o][:, :, p, :].rearrange("p t b -> p b t")
                    nc.vector.tensor_scalar_add(ov, sv, sc(bc + o))

            def conv6_k(k):
                for t in range(9):
                    nc.tensor.matmul(ps6[:], x6s[k][:, t, :, :],
                                     w6[k][:, t * 512:(t + 1) * 512],
                                     start=(k == 0 and t == 0),
                                     stop=(k == 3 and t == 8))

            conv5_oct(0)
            conv5_oct(1)
            conv6_k(0)
            conv5_oct(2)
            conv6_k(1)
            conv5_oct(3)
            conv6_k(2)
            conv6_k(3)
            y6a = sp.tile([32, 512], f32, tag="y6a")
            nc.vector.tensor_copy(y6a[:], ps6[0:32, :])
            y6 = sp.tile([32, 512], f32, tag="y6")
            nc.vector.tensor_tensor(y6[:], y6a[:], ps6[32:64, :], A.add)
            for k in range(4):
                pst = pt.tile([128, 32], f32, tag="pst")
                nc.tensor.transpose(pst[:], y6[:, k * 128:(k + 1) * 128], ident[:])
                nc.scalar.activation(h6[k][:], pst[:], SIGN,
                                     bias=sc(22 + k), scale=sc(18 + k))

        # ------------------------------------------------ fc1/fc2/fc3
        with ExitStack() as ctx:
            sp = ctx.enter_context(tc.tile_pool(name="sfc", bufs=1))
            pp = ctx.enter_context(tc.tile_pool(name="psf", bufs=3, space="PSUM"))
            for o in range(8):
                ps = pp.tile([128, B], f32, tag="ps")
                for k in range(4):
                    nc.tensor.matmul(ps[:], f1[k][:, o * 128:(o + 1) * 128], h6[k][:],
                                     start=(k == 0), stop=(k == 3))
                nc.vector.tensor_scalar(h7[o][:], ps[:], sc(26 + o), 2.0,
                                        A.is_ge, A.mult)
            for o in range(8):
                ps = pp.tile([128, B], f32, tag="ps")
                for k in range(8):
                    nc.tensor.matmul(ps[:], f2[k][:, o * 128:(o + 1) * 128], h7[k][:],
                                     start=(k == 0), stop=(k == 7))
                nc.vector.tensor_scalar(h8[o][:], ps[:], sc(42 + o), 2.0,
                                        A.is_ge, A.mult)
            ps = pp.tile([10, B], f32, tag="ps3")
            for k in range(8):
                nc.tensor.matmul(ps[:], f3[k][:], h8[k][:],
                                 start=(k == 0), stop=(k == 7))
            outsb = sp.tile([10, B], f32, tag="outsb")
            tmp9 = sp.tile([10, B], f32, tag="tmp9")
            nc.vector.tensor_scalar(tmp9[:], ps[:], bnv[0:10, 58:59], 2.0,
                                    A.is_ge, A.mult)
            nc.vector.tensor_scalar_sub(outsb[:], tmp9[:], 1.0)
            nc.sync.dma_start(od, outsb[:])

    nc.compile()
    return nc


_CACHE = {}


def _get_nc():
    if 'nc' not in _CACHE:
        _CACHE['nc'] = _build_nc()
    return _CACHE['nc']


def make_in_maps(**inputs):
    shared = _prep_shared(inputs)
    x = inputs['x'].astype(np.float32)
    in_maps = []
    for c in range(NCORES):
        m = dict(shared)
        m['xim'] = _prep_xim(x[c * B:(c + 1) * B])
        in_maps.append(m)
    return in_maps


def kernel(**inputs):
    from concourse.bass_utils import run_bass_kernel_spmd
    nc = _get_nc()
    in_maps = make_in_maps(**inputs)
    res = run_bass_kernel_spmd(nc, in_maps, core_ids=list(range(NCORES)))
    out = np.empty((NCORES * B, 10), np.float32)
    for c in range(NCORES):
        out[c * B:(c + 1) * B, :] = res.results[c]['od'].T
    return out


# revision 22
# speedup vs baseline: 1.0299x; 1.0299x over previous
"""Trainium2 Bass kernel for a binarized VGG-style CNN (BNN).

Data-parallel over 8 NeuronCores: batch 256 -> 32 per core; weights
replicated.  Host prep binarizes weights/activations and folds batch-
norms so every matmul is exact (products of +-1 / {0,2} are exact in
fp8/bf16/fp16, accumulation is fp32 in PSUM):

  conv1            : host im2col of sign(x), K=27, fp8, exact
  conv2/conv3      : fp8 {0,2} x fp8 +-1, 9-tap shifted GEMMs; vertical
                     tap pairs run in DoubleRow fp8 mode (row stride
                     padded to 16B so the j-step is legal); conv3 packs
                     3 batches per chunk by "rolling" rows across the
                     batch dim (seam rows computed then discarded)
  conv5            : im2col (X5col) fp8 DoubleRow over its 2 k-tiles
  activations      : {0,2} = sign+1 via one DVE is_ge per evict with
                     host-folded per-channel thresholds (conv3/conv5
                     thresholds are exact integers since actq precedes
                     the bn there)
  conv4            : input is bn3(sign); folded into weights w4*inv3,
                     split hi/lo fp16 (2 exact passes ~ 22 bits)
  conv6            : input bn5(sign) built on-chip in fp32, split in 2
                     fp16 parts, M-stacked (64 rows) flipped GEMM,
                     partials summed on DVE, PE-transposed back
  final evicts     : ScalarE Sign(x*scale + bias) (exact fp32 affine)

The PE is pre-warmed with dummy matmuls during the input DMA wait so
the HAM clock gate opens before conv1; bulky weight DMAs ride the
gpsimd SWDGE queues so the input im2col owns the HWDGE queue.
"""

import numpy as np
import ml_dtypes

BF = ml_dtypes.bfloat16
F16 = np.float16
F8 = ml_dtypes.float8_e4m3
NCORES = 8
B = 32
TAPS = [(kh, kw) for kh in range(3) for kw in range(3)]

# ----------------------------------------------------------------------------
# host-side prep
# ----------------------------------------------------------------------------

def _sign(a):
    return np.where(a >= 0, np.float32(1.0), np.float32(-1.0)).astype(np.float32)


def _inv_shift(bn):
    g, b, m, v = (bn[i].astype(np.float32) for i in range(4))
    inv = (g / np.sqrt(v + np.float32(1e-5))).astype(np.float32)
    shift = (b - m * inv).astype(np.float32)
    return inv, shift


def _split2_f16(w64):
    hi = w64.astype(F16)
    r = w64 - hi.astype(np.float64)
    lo = r.astype(F16)
    return hi, lo


def _prep_shared(i):
    d = {}
    inv1, shift1 = _inv_shift(i['bn1'])
    inv2, shift2 = _inv_shift(i['bn2'])
    inv3, shift3 = _inv_shift(i['bn3'])
    inv4, shift4 = _inv_shift(i['bn4'])
    inv5, shift5 = _inv_shift(i['bn5'])
    inv6, shift6 = _inv_shift(i['bn6'])
    inv7, shift7 = _inv_shift(i['bn7'])
    inv8, shift8 = _inv_shift(i['bn8'])
    inv9, shift9 = _inv_shift(i['bn9'])
    f64 = np.float64

    w1q = _sign(i['w1'])
    w1l = np.empty((27, 128), np.float32)
    for t, (kh, kw) in enumerate(TAPS):
        w1l[t * 3:(t + 1) * 3, :] = w1q[:, :, kh, kw].T
    d['w1d'] = w1l.astype(F8)

    # conv2 weights: per kw, DR pair (kh=0,1) interleaved + single kh=2
    #   col = kw*384 + j*128 + oc   (j = kh for the pair)
    #   col = kw*384 + 256 + oc     (kh = 2)
    w2q = _sign(i['w2'])
    w2l = np.empty((128, 3 * 384), np.float32)
    for kw in range(3):
        # DoubleRowSwInterleave storage: [A127,B127,A126,B126,...,A0,B0]
        # where A = kh0 weights, B = kh1 weights, column-reversed.
        for m in range(128):
            w2l[:, kw * 384 + 2 * (127 - m)] = w2q[m, :, 0, kw]
            w2l[:, kw * 384 + 2 * (127 - m) + 1] = w2q[m, :, 1, kw]
        w2l[:, kw * 384 + 256: kw * 384 + 384] = w2q[:, :, 2, kw].T
    d['w2d'] = w2l.astype(F8)

    # conv3 weights: plain lhsT per (tap, oct): col = (t*2+o)*128 + oc
    w3q = _sign(i['w3'])
    w3l = np.empty((128, 9 * 256), np.float32)
    for t, (kh, kw) in enumerate(TAPS):
        for o in range(2):
            w3l[:, (t * 2 + o) * 128:(t * 2 + o + 1) * 128] = \
                w3q[o * 128:(o + 1) * 128, :, kh, kw].T
    d['w3d'] = w3l.astype(F8)

    # conv4: folded weights w4q*inv3, 2-way fp16 split
    w4q = _sign(i['w4'])
    wt4 = w4q.astype(f64) * inv3.astype(f64)[None, :, None, None]
    hi, lo = _split2_f16(wt4)
    w4l = np.empty((256, 2 * 9 * 256), F16)
    for p, part in enumerate((hi, lo)):
        for t, (kh, kw) in enumerate(TAPS):
            for o in range(2):
                col = ((p * 9 + t) * 2 + o) * 128
                w4l[:, col:col + 128] = part[o * 128:(o + 1) * 128, :, kh, kw].T
    d['w4d'] = w4l

    # conv5 weights: DR over the 2 k-tiles: [c, j(kt), m] per (tap, oct)
    #   col = (t*4 + o)*256 + j*128 + m
    w5q = _sign(i['w5'])
    w5l = np.empty((128, 9 * 4 * 256), np.float32)
    for t, (kh, kw) in enumerate(TAPS):
        for o in range(4):
            base = (t * 4 + o) * 256
            for m in range(128):
                w5l[:, base + 2 * (127 - m)] = w5q[o * 128 + m, 0:128, kh, kw]
                w5l[:, base + 2 * (127 - m) + 1] = w5q[o * 128 + m, 128:256, kh, kw]
    d['w5d'] = w5l.astype(F8)

    w6q = _sign(i['w6'])
    w6r = np.empty((512, 9 * 512), np.float32)
    for t, (kh, kw) in enumerate(TAPS):
        w6r[:, t * 512:(t + 1) * 512] = w6q[:, :, kh, kw].T
    d['w6d'] = w6r.astype(BF)

    f1q = _sign(i['fc1'])
    f2q = _sign(i['fc2'])
    f3q = _sign(i['fc3'])
    d['f1d'] = f1q.T.copy().astype(BF)
    d['f2d'] = f2q.T.copy().astype(BF)
    d['f3d'] = f3q.T.copy().astype(BF)
    t7 = (-shift7.astype(f64) / inv7.astype(f64)).astype(np.float32)
    t8 = (f2q.sum(axis=1).astype(f64) - shift8.astype(f64) / inv8.astype(f64)
          ).astype(np.float32)
    t9 = (f3q.sum(axis=1).astype(f64) - shift9.astype(f64) / inv9.astype(f64)
          ).astype(np.float32)

    # thresholds ({0,2} propagation: psum = true_conv + rowsum(wq))
    rs2 = w2q.sum(axis=(1, 2, 3)).astype(f64)
    rs3 = w3q.sum(axis=(1, 2, 3)).astype(f64)
    rs5 = w5q.sum(axis=(1, 2, 3)).astype(f64)
    t1 = inv1  # ACT Sign scale
    t1b = shift1  # ACT Sign bias
    t2 = (-shift2.astype(f64) / inv2.astype(f64)).astype(np.float32)
    t3 = rs3.astype(np.float32)
    split_sum = hi.astype(f64) + lo.astype(f64)
    rowsum_split = split_sum.sum(axis=(1, 2, 3))
    shift3_term = w4q.sum(axis=(2, 3)).astype(f64) @ shift3.astype(f64)
    t4 = (rowsum_split - shift3_term - shift4.astype(f64) / inv4.astype(f64)
          ).astype(np.float32)
    t5 = rs5.astype(np.float32)

    bnv = np.zeros((128, 72), np.float32)
    bnv[:, 0] = t1
    bnv[:, 68] = t1b
    bnv[:, 1] = t2
    for o in range(2):
        bnv[:, 2 + o] = t3[o * 128:(o + 1) * 128]
        bnv[:, 4 + o] = t4[o * 128:(o + 1) * 128]
    # conv6 input X6 takes two values per channel: a = fp32(inv5+shift5)
    # (s5=+1) and b = fp32(-inv5+shift5).  Split each into fp16 hi+lo on
    # the host; on-chip the evict selects via (psum>=t5)*(hi_a-hi_b)+hi_b
    # (all arithmetic exact: fp16 diffs are exact in fp32, and the final
    # add reconstructs the fp16 value exactly).
    a = (inv5 + shift5).astype(np.float32)
    bq = (-inv5 + shift5).astype(np.float32)
    fa = a.astype(F16); fb = bq.astype(F16)
    la = (a.astype(f64) - fa.astype(f64)).astype(F16)
    lb = (bq.astype(f64) - fb.astype(f64)).astype(F16)
    dhi = (fa.astype(np.float32) - fb.astype(np.float32))
    dlo = (la.astype(np.float32) - lb.astype(np.float32))
    for k in range(4):
        bnv[:, 6 + k] = t5[k * 128:(k + 1) * 128]
        bnv[:, 10 + k] = dhi[k * 128:(k + 1) * 128]
        bnv[:, 14 + k] = fb.astype(np.float32)[k * 128:(k + 1) * 128]
        bnv[:, 60 + k] = dlo[k * 128:(k + 1) * 128]
        bnv[:, 64 + k] = lb.astype(np.float32)[k * 128:(k + 1) * 128]
        bnv[:, 18 + k] = inv6[k * 128:(k + 1) * 128]
        bnv[:, 22 + k] = shift6[k * 128:(k + 1) * 128]
    for o in range(8):
        bnv[:, 26 + o] = t7[o * 128:(o + 1) * 128]
        bnv[:, 42 + o] = t8[o * 128:(o + 1) * 128]
    bnv[:10, 58] = t9
    d['bnd'] = bnv
    return d


def _prep_xim(x_shard):
    """sign(x) im2col: [32,3,32,32] -> [27, 32*900] fp8."""
    xs = _sign(x_shard)
    arr = np.stack([xs[:, :, kh:kh + 30, kw:kw + 30] for (kh, kw) in TAPS], 0)
    return arr.transpose(0, 2, 1, 3, 4).reshape(27, 32 * 900).astype(F8)


# ----------------------------------------------------------------------------
# bass kernel build
# ----------------------------------------------------------------------------

def _build_nc():
    import concourse.bass as bass
    import concourse.bacc as bacc
    import concourse.tile as tile
    from concourse import mybir
    from concourse.masks import make_identity
    from contextlib import ExitStack

    bf16 = mybir.dt.bfloat16
    fp16 = mybir.dt.float16
    fp8 = mybir.dt.float8e4
    f32 = mybir.dt.float32
    A = mybir.AluOpType
    SIGN = mybir.ActivationFunctionType.Sign
    COPY = mybir.ActivationFunctionType.Identity
    DR = mybir.MatmulPerfMode.DoubleRow
    DRSW = mybir.MatmulPerfMode.DoubleRowSwInterleave

    nc = bacc.Bacc("TRN2", target_bir_lowering=False, debug=False)

    xim = nc.dram_tensor("xim", [27, B * 900], fp8, kind="ExternalInput").ap()
    w1d = nc.dram_tensor("w1d", [27, 128], fp8, kind="ExternalInput").ap()
    w2d = nc.dram_tensor("w2d", [128, 3 * 384], fp8, kind="ExternalInput").ap()
    w3d = nc.dram_tensor("w3d", [128, 9 * 256], fp8, kind="ExternalInput").ap()
    w4d = nc.dram_tensor("w4d", [256, 18 * 256], fp16, kind="ExternalInput").ap()
    w5d = nc.dram_tensor("w5d", [128, 9 * 4 * 256], fp8, kind="ExternalInput").ap()
    w6d = nc.dram_tensor("w6d", [512, 9 * 512], bf16, kind="ExternalInput").ap()
    f1d = nc.dram_tensor("f1d", [512, 1024], bf16, kind="ExternalInput").ap()
    f2d = nc.dram_tensor("f2d", [1024, 1024], bf16, kind="ExternalInput").ap()
    f3d = nc.dram_tensor("f3d", [1024, 10], bf16, kind="ExternalInput").ap()
    bnd = nc.dram_tensor("bnd", [128, 72], f32, kind="ExternalInput").ap()
    od = nc.dram_tensor("od", [10, B], f32, kind="ExternalOutput").ap()

    def sub_ap(base, extra_off, pairs):
        return bass.AP(base.tensor, base.offset + extra_off,
                       [list(base.ap[0])] + [list(p) for p in pairs])

    with tile.TileContext(nc) as tc, ExitStack() as top:
        const = top.enter_context(tc.tile_pool(name="const", bufs=1))
        bnv = const.tile([128, 72], f32, tag="bnv")
        nc.sync.dma_start(bnv[:], bnd)
        ident = const.tile([32, 32], f32, tag="ident")
        make_identity(nc, ident)

        def sc(c):
            return bnv[:, c:c + 1]

        acts = top.enter_context(tc.tile_pool(name="acts", bufs=1))
        h1 = acts.tile([128, B * 30 * 32], fp8, tag="h1")       # rows padded 30->32
        h2 = acts.tile([128, B * 14 * 16], fp8, tag="h2")       # rows padded 14->16
        s3 = [acts.tile([128, B * 144], fp8, name=f"s3_{k}", tag=f"s3_{k}")
              for k in range(2)]
        h4 = [acts.tile([128, B * 25], fp8, name=f"h4_{k}", tag=f"h4_{k}")
              for k in range(2)]
        x5c = acts.tile([128, 2, 9, B * 9], fp8, tag="x5c")
        x6s = [acts.tile([128, 9, 2, B], fp16, name=f"x6s_{k}", tag=f"x6s_{k}")
               for k in range(4)]
        h6 = [acts.tile([128, B], bf16, name=f"h6_{k}", tag=f"h6_{k}")
              for k in range(4)]
        h7 = [acts.tile([128, B], bf16, name=f"h7_{k}", tag=f"h7_{k}")
              for k in range(8)]
        h8 = [acts.tile([128, B], bf16, name=f"h8_{k}", tag=f"h8_{k}")
              for k in range(8)]

        # ---------------------------------------- PE pre-warm during DMA wait
        warm = top.enter_context(tc.tile_pool(name="warm", bufs=1))
        wz = warm.tile([128, 640], bf16, tag="wz")
        nc.gpsimd.memset(wz[:], 0.0)
        pwp_ctx = ExitStack()
        pwp = pwp_ctx.enter_context(tc.tile_pool(name="pswarm", bufs=1, space="PSUM"))
        pw = pwp.tile([128, 512], f32, tag="pw")
        _warm_n = [0]

        def dummy_mm(n=1, N=128):
            for _ in range(n):
                nc.tensor.matmul(pw[:, :N], wz[:, 0:128], wz[:, 128:128 + N],
                                 start=(_warm_n[0] == 0), stop=False,
                                 skip_group_check=True)
                _warm_n[0] += 1

        dummy_mm(26, N=512)
        pwp_ctx.close()

        # ---------------------------------------- weight DMAs
        # input path (sync / HWDGE queue): bnv, xim chunks, w1..w3
        # bulk weights (gpsimd / SWDGE queues): w4..w6, fc
        w2pool = top.enter_context(tc.tile_pool(name="w2pool", bufs=1))
        w3pool = top.enter_context(tc.tile_pool(name="w3pool", bufs=1))

        # -------------------------------- conv1 + conv2 (interleaved emission)
        # conv1 is K=27: on its own the PE activity monitor sees it as
        # near-idle and throttles the clock.  Emitting conv1(b) interleaved
        # with conv2(b-1) keeps dense K=128 DR work in the PE stream, so
        # the whole phase runs at full clock.
        w4pool = top.enter_context(tc.tile_pool(name="w4pool", bufs=1))
        w4 = [w4pool.tile([128, 18 * 256], fp16, name=f"w4_{k}", tag=f"w4_{k}")
              for k in range(2)]
        _w4dmas = [nc.gpsimd.dma_start(w4[k][:], w4d[k * 128:(k + 1) * 128, :])
                   for k in range(2)]
        with ExitStack() as ctx:
            p01 = ctx.enter_context(tc.tile_pool(name="p01", bufs=1))
            ximt = p01.tile([27, B * 900], fp8, tag="ximt")
            xdma = None
            for q in range(8):
                s = q * 4 * 900
                e = (q + 1) * 4 * 900
                eng = nc.sync if q % 2 == 0 else nc.scalar
                xdma = eng.dma_start(ximt[:, s:e], xim[:, s:e])
            w1 = p01.tile([27, 128], fp8, tag="w1")
            nc.sync.dma_start(w1[:], w1d)
            w2 = w2pool.tile([128, 3 * 384], fp8, tag="w2")
            nc.sync.dma_start(w2[:], w2d)
            w3 = w3pool.tile([128, 9 * 256], fp8, tag="w3")
            nc.sync.dma_start(w3[:], w3d)
            from concourse.tile_rust import add_dep_helper
            for _wd in _w4dmas:
                add_dep_helper(_wd.ins, xdma.ins, sync=True,
                               reason="bulk weight DMA waits for input im2col")
            pp1 = ctx.enter_context(tc.tile_pool(name="ps1", bufs=2, space="PSUM"))
            sp = ctx.enter_context(tc.tile_pool(name="sc2", bufs=4))
            pp2 = ctx.enter_context(tc.tile_pool(name="ps2", bufs=6, space="PSUM"))
            xr = ximt[:].rearrange("p (b h w) -> p b h w", b=B, h=30, w=30)
            h1v = h1[:].rearrange("p (b h w) -> p b h w", b=B, h=30, w=32)
            nc.gpsimd.memset(h1v[:, :, :, 30:32], 0.0)
            h2v = h2[:].rearrange("p (b h w) -> p b h w", b=B, h=14, w=16)
            NR = 13 * 32 + 28

            def conv1_emit(b):
                for rh in range(2):
                    ps = pp1.tile([128, 450], f32, tag="ps1t")
                    nc.tensor.matmul(ps[:], w1[:], xr[:, b, rh * 15:rh * 15 + 15, :],
                                     start=True, stop=True)
                    psv = ps[:].rearrange("p (r w) -> p r w", r=15, w=30)
                    nc.scalar.activation(
                        h1v[:, b, rh * 15:(rh + 1) * 15, 0:30],
                        psv, SIGN, bias=sc(68), scale=sc(0))

            def conv2_emit(bpair):
                chunks = [(b, rh) for b in bpair for rh in range(2)]
                # 2-chunk groups measured faster than 4
                pss = [pp2.tile([128, NR], f32, name=f"ps_{b}_{rh}", tag="ps")
                       for (b, rh) in chunks]
                for kw in range(3):
                    for ci, (b, rh) in enumerate(chunks):
                        rhs = sub_ap(h1[:], b * 960 + rh * 14 * 32 + kw,
                                     [[32, 2], [1, NR]])
                        nc.tensor.matmul(
                            pss[ci][:], w2[:, kw * 384:kw * 384 + 256],
                            rhs, start=(kw == 0), stop=False, perf_mode=DRSW)
                    for ci, (b, rh) in enumerate(chunks):
                        rhs2 = sub_ap(h1[:], b * 960 + (rh * 14 + 2) * 32 + kw,
                                      [[1, NR]])
                        nc.tensor.matmul(
                            pss[ci][:], w2[:, kw * 384 + 256:kw * 384 + 384],
                            rhs2, start=False, stop=(kw == 2))
                for ci, (b, rh) in enumerate(chunks):
                    ps = pss[ci]
                    tmp = sp.tile([128, 392], bf16, tag="tmp")
                    psv = sub_ap(ps[:], 0, [[32, 14], [1, 28]])
                    tmpv = tmp[:].rearrange("p (r w) -> p r w", r=14, w=28)
                    nc.vector.tensor_scalar(tmpv, psv, sc(1), 2.0, A.is_ge, A.mult)
                    tr = tmp[:].rearrange("p (r rr w ww) -> p r rr w ww",
                                          r=7, rr=2, w=14, ww=2)
                    m1 = sp.tile([128, 98], bf16, tag="m1")
                    m1r = m1[:].rearrange("p (r w) -> p r w", r=7, w=14)
                    m2 = sp.tile([128, 98], bf16, tag="m2")
                    m2r = m2[:].rearrange("p (r w) -> p r w", r=7, w=14)
                    nc.vector.tensor_tensor(m1r, tr[:, :, 0, :, 0], tr[:, :, 0, :, 1], A.max)
                    nc.vector.tensor_tensor(m2r, tr[:, :, 1, :, 0], tr[:, :, 1, :, 1], A.max)
                    nc.vector.tensor_tensor(
                        h2v[:, b, rh * 7:(rh + 1) * 7, 0:14], m1r, m2r, A.max)

            for b in range(B):
                conv1_emit(b)
                if b >= 1:
                    conv2_emit((b - 1,))
            conv2_emit((B - 1,))

        # ------------------------------------------------ conv3 -> s3 {0,2}
        bgrp3 = [(b0, min(b0 + 3, B)) for b0 in range(0, B, 3)]
        with ExitStack() as ctx:
            w5pool = top.enter_context(tc.tile_pool(name="w5pool", bufs=1))
            w5 = w5pool.tile([128, 9 * 4 * 256], fp8, tag="w5")
            nc.gpsimd.dma_start(w5[:], w5d)
            pp = ctx.enter_context(tc.tile_pool(name="ps3", bufs=6, space="PSUM"))
            h2v3 = h2[:].rearrange("p (b h w) -> p b h w", b=B, h=14, w=16)
            for o in range(2):
                for (b0, b1) in bgrp3:
                    nb = b1 - b0
                    ps = pp.tile([128, 3 * 144], f32, tag="ps")
                    for t, (kh, kw) in enumerate(TAPS):
                        nc.tensor.matmul(
                            ps[:, :nb * 144],
                            w3[:, (t * 2 + o) * 128:(t * 2 + o + 1) * 128],
                            h2v3[:, b0:b1, kh:kh + 12, kw:kw + 12],
                            start=(t == 0), stop=(t == 8))
                    nc.vector.tensor_scalar(
                        s3[o][:, b0 * 144:b1 * 144], ps[:, :nb * 144],
                        sc(2 + o), 2.0, A.is_ge, A.mult)

        # ------------------------------------------------ conv4 (2-pass fp16)
        bgrp4 = [(b0, min(b0 + 5, B)) for b0 in range(0, B, 5)]
        with ExitStack() as ctx:
            w6pool = top.enter_context(tc.tile_pool(name="w6pool", bufs=1))
            w6 = [w6pool.tile([128, 9 * 512], bf16, name=f"w6_{k}", tag=f"w6_{k}")
                  for k in range(4)]
            for k in range(4):
                nc.gpsimd.dma_start(w6[k][:], w6d[k * 128:(k + 1) * 128, :])
            fcpool = top.enter_context(tc.tile_pool(name="fcpool", bufs=1))
            f1 = [fcpool.tile([128, 1024], bf16, name=f"f1_{k}", tag=f"f1_{k}")
                  for k in range(4)]
            f2 = [fcpool.tile([128, 1024], bf16, name=f"f2_{k}", tag=f"f2_{k}")
                  for k in range(8)]
            f3 = [fcpool.tile([128, 10], bf16, name=f"f3_{k}", tag=f"f3_{k}")
                  for k in range(8)]
            for k in range(4):
                nc.gpsimd.dma_start(f1[k][:], f1d[k * 128:(k + 1) * 128, :])
            for k in range(8):
                nc.gpsimd.dma_start(f2[k][:], f2d[k * 128:(k + 1) * 128, :])
            for k in range(8):
                nc.gpsimd.dma_start(f3[k][:], f3d[k * 128:(k + 1) * 128, :])
            sp = ctx.enter_context(tc.tile_pool(name="sc4", bufs=4))
            pp = ctx.enter_context(tc.tile_pool(name="ps4", bufs=6, space="PSUM"))
            s3v = [s3[k][:].rearrange("p (b h w) -> p b h w", b=B, h=12, w=12)
                   for k in range(2)]
            h4v = [h4[k][:].rearrange("p (b h w) -> p b h w", b=B, h=5, w=5)
                   for k in range(2)]
            for o in range(2):
                for (b0, b1) in bgrp4:
                    nb = b1 - b0
                    n = nb * 100
                    ps = pp.tile([128, 500], f32, tag="ps")
                    first = True
                    for p in range(2):
                        for t, (kh, kw) in enumerate(TAPS):
                            for k in range(2):
                                col = ((p * 9 + t) * 2 + o) * 128
                                nc.tensor.matmul(
                                    ps[:, :n], w4[k][:, col:col + 128],
                                    s3v[k][:, b0:b1, kh:kh + 10, kw:kw + 10],
                                    start=first, stop=(p == 1 and t == 8 and k == 1))
                                first = False
                    tmp = sp.tile([128, 500], bf16, tag="tmp")
                    nc.vector.tensor_scalar(tmp[:, :n], ps[:, :n],
                                            sc(4 + o), 2.0, A.is_ge, A.mult)
                    tr = tmp[:, :n].rearrange("p (b r rr w ww) -> p b r rr w ww",
                                              b=nb, r=5, rr=2, w=5, ww=2)
                    m1 = sp.tile([128, 125], bf16, tag="m1")
                    m1r = m1[:, :nb * 25].rearrange("p (b r w) -> p b r w", b=nb, r=5, w=5)
                    m2 = sp.tile([128, 125], bf16, tag="m2")
                    m2r = m2[:, :nb * 25].rearrange("p (b r w) -> p b r w", b=nb, r=5, w=5)
                    nc.vector.tensor_tensor(m1r, tr[:, :, :, 0, :, 0], tr[:, :, :, 0, :, 1], A.max)
                    nc.vector.tensor_tensor(m2r, tr[:, :, :, 1, :, 0], tr[:, :, :, 1, :, 1], A.max)
                    nc.vector.tensor_tensor(h4v[o][:, b0:b1, :, :], m1r, m2r, A.max)

        # ------------------------------------------------ conv5 (X5col fp8 DR)
        with ExitStack() as ctx:
            sp = ctx.enter_context(tc.tile_pool(name="sc5", bufs=2))
            pp = ctx.enter_context(tc.tile_pool(name="ps5", bufs=4, space="PSUM"))
            h4r = [h4[k][:].rearrange("p (b h w) -> p b h w", b=B, h=5, w=5)
                   for k in range(2)]
            for j in range(2):
                for t, (kh, kw) in enumerate(TAPS):
                    ov = x5c[:, j, t, :].rearrange("p (b h w) -> p b h w",
                                                   b=B, h=3, w=3)
                    nc.vector.tensor_copy(ov, h4r[j][:, :, kh:kh + 3, kw:kw + 3])
            pp6 = ctx.enter_context(tc.tile_pool(name="ps6", bufs=1, space="PSUM"))
            pt = ctx.enter_context(tc.tile_pool(name="ps6t", bufs=2, space="PSUM"))
            ps6 = pp6.tile([64, 512], f32, tag="ps6")

            def conv5_oct(o):
                ps = pp.tile([128, B * 9], f32, tag="ps")
                for t in range(9):
                    nc.tensor.matmul(
                        ps[:], w5[:, (t * 4 + o) * 256:(t * 4 + o + 1) * 256],
                        x5c[:, :, t, :], start=(t == 0), stop=(t == 8),
                        perf_mode=DRSW)
                for p, (dc, bc) in enumerate(((10, 14), (60, 64))):
                    sel = sp.tile([128, B * 9], f32, tag="sel")
                    nc.vector.tensor_scalar(sel[:], ps[:], sc(6 + o), sc(dc + o),
                                            A.is_ge, A.mult)
                    sv = sel[:].rearrange("p (b t) -> p b t", b=B, t=9)
                    ov = x6s[o][:, :, p, :].rearrange("p t b -> p b t")
                    nc.vector.tensor_scalar_add(ov, sv, sc(bc + o))

            def conv6_k(k):
                for t in range(9):
                    nc.tensor.matmul(ps6[:], x6s[k][:, t, :, :],
                                     w6[k][:, t * 512:(t + 1) * 512],
                                     start=(k == 0 and t == 0),
                                     stop=(k == 3 and t == 8))

            conv5_oct(0)
            conv5_oct(1)
            conv6_k(0)
            conv5_oct(2)
            conv6_k(1)
            conv5_oct(3)
            conv6_k(2)
            conv6_k(3)
            y6a = sp.tile([32, 512], f32, tag="y6a")
            nc.vector.tensor_copy(y6a[:], ps6[0:32, :])
            y6 = sp.tile([32, 512], f32, tag="y6")
            nc.vector.tensor_tensor(y6[:], y6a[:], ps6[32:64, :], A.add)
            for k in range(4):
                pst = pt.tile([128, 32], f32, tag="pst")
                nc.tensor.transpose(pst[:], y6[:, k * 128:(k + 1) * 128], ident[:])
                nc.scalar.activation(h6[k][:], pst[:], SIGN,
                                     bias=sc(22 + k), scale=sc(18 + k))

        # ------------------------------------------------ fc1/fc2/fc3
        with ExitStack() as ctx:
            sp = ctx.enter_context(tc.tile_pool(name="sfc", bufs=1))
            pp = ctx.enter_context(tc.tile_pool(name="psf", bufs=3, space="PSUM"))
            for o in range(8):
                ps = pp.tile([128, B], f32, tag="ps")
                for k in range(4):
                    nc.tensor.matmul(ps[:], f1[k][:, o * 128:(o + 1) * 128], h6[k][:],
                                     start=(k == 0), stop=(k == 3))
                nc.vector.tensor_scalar(h7[o][:], ps[:], sc(26 + o), 2.0,
                                        A.is_ge, A.mult)
            for o in range(8):
                ps = pp.tile([128, B], f32, tag="ps")
                for k in range(8):
                    nc.tensor.matmul(ps[:], f2[k][:, o * 128:(o + 1) * 128], h7[k][:],
                                     start=(k == 0), stop=(k == 7))
                nc.vector.tensor_scalar(h8[o][:], ps[:], sc(42 + o), 2.0,
                                        A.is_ge, A.mult)
            ps = pp.tile([10, B], f32, tag="ps3")
            for k in range(8):
                nc.tensor.matmul(ps[:], f3[k][:], h8[k][:],
                                 start=(k == 0), stop=(k == 7))
            outsb = sp.tile([10, B], f32, tag="outsb")
            tmp9 = sp.tile([10, B], f32, tag="tmp9")
            nc.vector.tensor_scalar(tmp9[:], ps[:], bnv[0:10, 58:59], 2.0,
                                    A.is_ge, A.mult)
            nc.vector.tensor_scalar_sub(outsb[:], tmp9[:], 1.0)
            nc.sync.dma_start(od, outsb[:])

    nc.compile()
    return nc


_CACHE = {}


def _get_nc():
    if 'nc' not in _CACHE:
        _CACHE['nc'] = _build_nc()
    return _CACHE['nc']


def make_in_maps(**inputs):
    shared = _prep_shared(inputs)
    x = inputs['x'].astype(np.float32)
    in_maps = []
    for c in range(NCORES):
        m = dict(shared)
        m['xim'] = _prep_xim(x[c * B:(c + 1) * B])
        in_maps.append(m)
    return in_maps


def kernel(**inputs):
    from concourse.bass_utils import run_bass_kernel_spmd
    nc = _get_nc()
    in_maps = make_in_maps(**inputs)
    res = run_bass_kernel_spmd(nc, in_maps, core_ids=list(range(NCORES)))
    out = np.empty((NCORES * B, 10), np.float32)
    for c in range(NCORES):
        out[c * B:(c + 1) * B, :] = res.results[c]['od'].T
    return out
